# revision 23
# baseline (speedup 1.0000x reference)
"""Trainium2 Bass kernel for the shared-weight transformer encoder with a
Conv1d-ensemble FFN (nn_MCAT_23630910062939).

Sharding: data-parallel over batch — each of the 8 NeuronCores computes one
full batch element; no collectives.  The residual stream lives on-chip in
feature-major layout x^T [D, S] (bf16); host transposes input/output.

Precision plan (validated against a numpy error model, all-batch max rel
6.4e-3 vs the 2e-2 gate): the conv ensemble (60% of the MACs) runs in bf16 —
fp8 there blows the error budget.  The attention-side projections (Q, K, V,
output) run in fp8e4m3 DoubleRow mode (2 rows/cycle); their weights are
pre-scaled by 64 on the host and descaled in the PSUM-drain activation.
DoubleRow outputs must land on PSUM partitions 0-63 (s3d3 ISA rule), so
projections use two 64-row psum tiles per 128-feature chunk.
"""
import sys, os
sys.path.insert(0, '/opt/trn_rl_repo')
import numpy as np
import ml_dtypes

from contextlib import ExitStack
import concourse.bass as bass
import concourse.mybir as mybir
import concourse.tile as tile
from concourse import bacc, library_config
from concourse.bass_utils import run_bass_kernel_spmd

P = 128
D = 1024
S = 1024
H = 16
DK = 64
CH = D // P          # 8 feature chunks
KP = CH // 2         # 4 DoubleRow k-tile pairs
NH = 2               # 512-wide halves of the token axis
NL = 2               # shared layer applied twice
N_CORES = 8
EPS_LN = 1e-6
EPS_BN = 1e-5
WSCALE = 64.0        # fp8 weight pre-scale (w sigma 0.02 -> 1.3)

f32 = mybir.dt.float32
f32r = mybir.dt.float32r
bf16 = mybir.dt.bfloat16
fp8 = mybir.dt.float8e4
AF = mybir.ActivationFunctionType
OP = mybir.AluOpType
PM = mybir.MatmulPerfMode

# conv taps in pack order: (pack_idx, branch, shift)
# branch 0: filter 5 (shifts -2..2), branch 1: filter 3 (-1..1), branch 2: filter 1 (0)
BRANCH_TAPS = [
    [(0, -2), (1, -1), (2, 0), (3, 1), (4, 2)],
    [(5, -1), (6, 0), (7, 1)],
    [(8, 0)],
]
N_TAPS = 9


def _build():
    nc = bacc.Bacc(None, target_bir_lowering=False)
    names = {}

    def reg(t, key):
        names[key] = t.name
        return t

    with tile.TileContext(nc) as tc, ExitStack() as stack:
        with tc.tile_pool(name="dram", bufs=1, space="DRAM") as dram:
            xt_d = reg(dram.tile([D, S], bf16, kind="ExternalInput", name="xt"), "xt")
            # fp8 DoubleRow weight blocks: [m-chunk, partition(k%128), kpair, j, out(128)]
            wq_d = reg(dram.tile([CH, P, KP, 2, P], fp8, kind="ExternalInput", name="wq"), "wq")
            wk_d = reg(dram.tile([CH, P, KP, 2, P], fp8, kind="ExternalInput", name="wk"), "wk")
            wo_d = reg(dram.tile([CH, P, KP, 2, P], fp8, kind="ExternalInput", name="wo"), "wo")
            wv_d = reg(dram.tile([CH, P, S], fp8, kind="ExternalInput", name="wv"), "wv")
            cw_d = reg(dram.tile([N_TAPS, CH, P, CH, P], bf16, kind="ExternalInput", name="cw"), "cw")
            uw_d = reg(dram.tile([8, CH, P, CH, P], bf16, kind="ExternalInput", name="uw"), "uw")
            bq_d = reg(dram.tile([P, CH], f32, kind="ExternalInput", name="bq"), "bq")
            bk_d = reg(dram.tile([P, CH], f32, kind="ExternalInput", name="bk"), "bk")
            bo_d = reg(dram.tile([P, CH], f32, kind="ExternalInput", name="bo"), "bo")
            bv_d = reg(dram.tile([1, S], f32, kind="ExternalInput", name="bv"), "bv")
            cb_d = reg(dram.tile([P, 3, CH], f32, kind="ExternalInput", name="cb"), "cb")
            yt_d = reg(dram.tile([D, S], bf16, kind="ExternalOutput", name="yt"), "yt")

            nc.gpsimd.load_library(library_config.attn)

            const = stack.enter_context(tc.tile_pool(name="const", bufs=1))
            ones32 = const.tile([P, 1], f32)
            nc.vector.memset(ones32[:], 1.0 / D)
            ones_b = const.tile([P, 1], bf16)
            nc.vector.tensor_copy(ones_b[:], ones32[:])
            bq_sb = const.tile([P, CH], f32)
            bk_sb = const.tile([P, CH], f32)
            bo_sb = const.tile([P, CH], f32)
            cb_sb = const.tile([P, 3, CH], f32)
            nc.sync.dma_start(bq_sb[:], bq_d[:])
            nc.sync.dma_start(bk_sb[:], bk_d[:])
            nc.sync.dma_start(bo_sb[:], bo_d[:])
            nc.sync.dma_start(cb_sb[:], cb_d[:])
            bv_row = const.tile([1, S], f32)
            nc.sync.dma_start(bv_row[:], bv_d[:])
            bv_b = const.tile([P, S], f32)
            nc.gpsimd.partition_broadcast(bv_b[:], bv_row[:])

            glob = stack.enter_context(tc.tile_pool(name="glob", bufs=1))
            x = glob.tile([P, CH, S], bf16, tag="x")          # residual x^T
            act8 = glob.tile([P, CH, S], fp8, tag="act8")     # LN1 output h (fp8)
            g = glob.tile([P, CH, S + 4], bf16, tag="g")      # LN2 output (padded)
            vw8 = glob.tile([P, CH, S], fp8, tag="vw8")       # V weights (resident)
            z = glob.tile([P, CH, 4, 512], bf16, tag="z")     # F(2,3) input transform
            for c in range(CH):
                nc.sync.dma_start(x[:, c, :], xt_d[c * P:(c + 1) * P, :])
            for c in range(CH):
                nc.sync.dma_start(vw8[:, c, :], wv_d[c])
            z16 = const.tile([P, 16], bf16)
            nc.vector.memset(z16[:], 0.0)
            zv = z16[:].rearrange("p (c e) -> p c e", e=2)
            nc.vector.tensor_copy(g[:, :, 0:2], zv)
            nc.vector.tensor_copy(g[:, :, S + 2:S + 4], zv)

            def emit_ln(dst, dst_off, sq_on_act=True):
                """LayerNorm over the feature (partition) axis of x.
                dst: act8 (fp8, attention input), g (bf16, conv input, +2 col
                offset), or None (final LN -> DMA f32 to yt_d)."""
                with tc.tile_pool(name="lnps", bufs=1, space="PSUM") as lnps, \
                     tc.tile_pool(name="lnsb", bufs=1) as lnsb:
                    mean_ps = lnps.tile([1, S], f32, tag="mean")
                    msq_ps = lnps.tile([1, S], f32, tag="msq")
                    for c in range(CH):
                        sq = lnsb.tile([P, S], bf16, tag="sq", bufs=2)
                        if sq_on_act:
                            nc.scalar.activation(sq[:], x[:, c, :], AF.Square)
                        else:
                            nc.vector.tensor_mul(sq[:], x[:, c, :], x[:, c, :])
                        for n in range(NH):
                            sl = slice(n * 512, (n + 1) * 512)
                            nc.tensor.matmul(mean_ps[:, sl], ones_b[:], x[:, c, sl],
                                             start=(c == 0), stop=(c == CH - 1))
                            nc.tensor.matmul(msq_ps[:, sl], ones_b[:], sq[:, sl],
                                             start=(c == 0), stop=(c == CH - 1))
                    lp = nc.allow_low_precision(reason="LN stats rows in bf16; 0.4% std err ok")
                    lp.__enter__()
                    mean_sb = lnsb.tile([1, S], bf16, tag="mrow")
                    nc.vector.tensor_copy(mean_sb[:], mean_ps[:])
                    m2 = lnsb.tile([1, S], bf16, tag="m2")
                    nc.vector.tensor_mul(m2[:], mean_sb[:], mean_sb[:])
                    var0 = lnsb.tile([1, S], bf16, tag="var0")
                    nc.vector.tensor_tensor(var0[:], msq_ps[:], m2[:], OP.subtract)
                    # 1/(unbiased std) = (var0 * D/(D-1))^-0.5  (eps ~ 1e-6
                    # is immaterial at our error scale): one DVE op,
                    # out = pow(var0, -0.5) * sqrt((D-1)/D)
                    inv_rb = lnsb.tile([1, S], bf16, tag="invrb")
                    nc.vector.tensor_scalar(inv_rb[:], var0[:], -0.5,
                                            float((D - 1.0) / D) ** 0.5,
                                            op0=OP.pow, op1=OP.mult)
                    minv_rb = lnsb.tile([1, S], bf16, tag="minvrb")
                    nc.vector.tensor_mul(minv_rb[:], mean_sb[:], inv_rb[:])
                    inv_b = lnsb.tile([P, S], bf16, tag="invb")
                    minv_b = lnsb.tile([P, S], bf16, tag="minvb")
                    nc.gpsimd.partition_broadcast(inv_b[:], inv_rb[:])
                    nc.gpsimd.partition_broadcast(minv_b[:], minv_rb[:])
                    lp.__exit__(None, None, None)
                    for c in range(CH):
                        t = lnsb.tile([P, S], bf16, tag="lnapp", bufs=2)
                        nc.vector.tensor_mul(t[:], x[:, c, :], inv_b[:])
                        if dst is not None:
                            nc.vector.tensor_tensor(dst[:, c, dst_off:dst_off + S],
                                                    t[:], minv_b[:], OP.subtract)
                        else:
                            o = lnsb.tile([P, S], bf16, tag="lnout", bufs=2)
                            nc.vector.tensor_tensor(o[:], t[:], minv_b[:], OP.subtract)
                            nc.sync.dma_start(yt_d[c * P:(c + 1) * P, :], o[:])

            for layer in range(NL):
                # ---------------- LN1 -> h (act8, fp8) ----------------
                emit_ln(act8, 0)

                with tc.tile_pool(name="attnbuf", bufs=1) as ab:
                    qt = ab.tile([P, CH, S], bf16, tag="qt")
                    kt = ab.tile([P, CH, S], bf16, tag="kt")
                    vt = ab.tile([P, CH, H * 65], bf16, tag="vt")
                    ot = ab.tile([P, CH, S], fp8, tag="ot")

                    # ---------------- Q,K projections (fp8 DoubleRow) --------
                    # Q/K interleaved per feature-chunk m so attention head
                    # pair 2m,2m+1 unblocks right after chunk m; Q drains on
                    # ACT, K on DVE (tensor_scalar) to split the drain load.
                    with tc.tile_pool(name="qkps", bufs=1, space="PSUM") as qkps, \
                         tc.tile_pool(name="qksb", bufs=1) as qksb:
                        for m in range(CH):
                            for (w_d, bsb, dst, on_act) in ((wq_d, bq_sb, qt, True),
                                                            (wk_d, bk_sb, kt, False)):
                                wblk = qksb.tile([P, KP, 2, P], fp8, tag="wproj", bufs=4)
                                nc.sync.dma_start(wblk[:], w_d[m])
                                for mb in range(2):
                                    msl = slice(mb * 64, (mb + 1) * 64)
                                    ps = qkps.tile([64, S], f32, tag=f"pps{mb}", bufs=1)
                                    for kp in range(KP):
                                        for n in range(NH):
                                            nc.tensor.matmul(
                                                ps[:, n * 512:(n + 1) * 512],
                                                wblk[:, kp, :, msl],
                                                act8[:, 2 * kp:2 * kp + 2, n * 512:(n + 1) * 512],
                                                start=(kp == 0), stop=(kp == KP - 1),
                                                perf_mode=PM.DoubleRow)
                                    if on_act:
                                        nc.scalar.activation(dst[msl, m, :], ps[:], AF.Identity,
                                                             bias=bsb[msl, m:m + 1], scale=1.0 / WSCALE)
                                    else:
                                        nc.vector.tensor_scalar(dst[msl, m, :], ps[:],
                                                                1.0 / WSCALE, bsb[msl, m:m + 1],
                                                                op0=OP.mult, op1=OP.add)

                        # ---------- V projection (fp8 DoubleRow, token-major)
                        nc.vector.memset(vt[:].rearrange("p c (h e) -> p c h e", e=65)[:, :, :, 64:65], 1.0)
                        for tb in range(2 * CH):
                            pv = qkps.tile([64, S], f32, tag="pv", bufs=2)
                            for kp in range(KP):
                                for n in range(NH):
                                    nc.tensor.matmul(
                                        pv[:, n * 512:(n + 1) * 512],
                                        act8[:, 2 * kp:2 * kp + 2, tb * 64:(tb + 1) * 64],
                                        vw8[:, 2 * kp:2 * kp + 2, n * 512:(n + 1) * 512],
                                        start=(kp == 0), stop=(kp == KP - 1),
                                        perf_mode=PM.DoubleRow)
                            p0 = (tb % 2) * 64
                            dstv = vt[p0:p0 + 64, tb // 2, :].rearrange("p (h e) -> p h e", e=65)[:, :, 0:64]
                            src = pv[:].rearrange("p (h e) -> p h e", e=64)
                            bvv = bv_b[0:64, :].rearrange("p (h e) -> p h e", e=64)
                            nc.vector.scalar_tensor_tensor(dstv, src, 1.0 / WSCALE, bvv,
                                                           op0=OP.mult, op1=OP.add)

                    # ---------------- attention per head ----------------
                    with tc.tile_pool(name="atps", bufs=1, space="PSUM") as atps, \
                         tc.tile_pool(name="atsb", bufs=1) as atsb:
                        for h in range(H):
                            hp, off = h // 2, 64 * (h % 2)
                            pexp = atsb.tile([P, CH, S], bf16, tag="pexp", bufs=2)
                            for mk in range(CH):
                                scps = atps.tile([P, S], f32, tag="scps", bufs=2)
                                for n in range(NH):
                                    nc.tensor.matmul(scps[:, n * 512:(n + 1) * 512],
                                                     kt[off:off + 64, hp, mk * P:(mk + 1) * P],
                                                     qt[off:off + 64, hp, n * 512:(n + 1) * 512],
                                                     start=True, stop=True)
                                nc.scalar.activation(pexp[:, mk, :], scps[:], AF.Exp)
                            ops = atps.tile([65, S], f32, tag="ops", bufs=2)
                            for c in range(CH):
                                for n in range(NH):
                                    sl = slice(n * 512, (n + 1) * 512)
                                    nc.tensor.matmul(ops[:, sl], vt[:, c, 65 * h:65 * h + 65],
                                                     pexp[:, c, sl],
                                                     start=(c == 0), stop=(c == CH - 1))
                            rrow = atsb.tile([1, S], f32, tag="rrow", bufs=2)
                            nc.vector.reciprocal(rrow[:], ops[64:65, :])
                            rb = atsb.tile([64, S], f32, tag="rb", bufs=2)
                            nc.gpsimd.partition_broadcast(rb[:], rrow[:])
                            nc.vector.tensor_tensor(ot[off:off + 64, hp, :], ops[0:64, :], rb[:], OP.mult)

                    # ---------------- output projection + residual ----------
                    with tc.tile_pool(name="wops", bufs=1, space="PSUM") as wops, \
                         tc.tile_pool(name="wosb", bufs=1) as wosb:
                        for m in range(CH):
                            woblk = wosb.tile([P, KP, 2, P], fp8, tag="wo", bufs=3)
                            nc.sync.dma_start(woblk[:], wo_d[m])
                            att = wosb.tile([P, S], bf16, tag="att", bufs=2)
                            for mb in range(2):
                                msl = slice(mb * 64, (mb + 1) * 64)
                                ps = wops.tile([64, S], f32, tag=f"ops{mb}", bufs=2)
                                for kp in range(KP):
                                    for n in range(NH):
                                        nc.tensor.matmul(
                                            ps[:, n * 512:(n + 1) * 512],
                                            woblk[:, kp, :, msl],
                                            ot[:, 2 * kp:2 * kp + 2, n * 512:(n + 1) * 512],
                                            start=(kp == 0), stop=(kp == KP - 1),
                                            perf_mode=PM.DoubleRow)
                                nc.scalar.activation(att[msl, :], ps[:], AF.Identity,
                                                     bias=bo_sb[msl, m:m + 1], scale=1.0 / WSCALE)
                            nc.vector.tensor_tensor(x[:, m, :], x[:, m, :], att[:], OP.add)

                # ---------------- LN2 -> g (bf16, padded) ----------------
                emit_ln(g, 2, sq_on_act=False)

                # F(2,3) input transform for the 3-tap branch: for output pair
                # (2u, 2u+1): d = g(2u-1 .. 2u+2); z0 = d0-d2, z1 = d1+d2,
                # z2 = d2-d1, z3 = d1-d3.  view(a) = g cols a+2t (t in 0..511).
                for c in range(CH):
                    view = lambda a: g[:, c, a:a + 1024].rearrange(
                        "p (t e) -> p e t", e=2)[:, 0, :]
                    nc.vector.tensor_tensor(z[:, c, 0, :], view(1), view(3), OP.subtract)
                    nc.vector.tensor_tensor(z[:, c, 1, :], view(2), view(3), OP.add)
                    nc.vector.tensor_tensor(z[:, c, 2, :], view(3), view(2), OP.subtract)
                    nc.vector.tensor_tensor(z[:, c, 3, :], view(2), view(4), OP.subtract)

                # ---------------- conv ensemble FFN (bf16) -------
                with tc.tile_pool(name="cvps", bufs=1, space="PSUM") as cvps, \
                     tc.tile_pool(name="cvsb", bufs=1) as cvsb:
                    def wino(wps, m, base, dst_even, dst_odd):
                        # phases m_i = U_i^T z_i; y_even = m0+m1+m2, y_odd = m1-m2-m3
                        for i in range(4):
                            uwblk = cvsb.tile([P, CH, P], bf16, tag="uw", bufs=6)
                            nc.sync.dma_start(uwblk[:], uw_d[base + i, m])
                            for c in range(CH):
                                nc.tensor.matmul(wps[:, i, :], uwblk[:, c, :],
                                                 z[:, c, i, :],
                                                 start=(c == 0), stop=(c == CH - 1))
                        # DVE may read only one PSUM operand per op: stage m1
                        m1 = cvsb.tile([P, 512], bf16, tag="m1s", bufs=2)
                        nc.vector.tensor_copy(m1[:], wps[:, 1, :])
                        te = cvsb.tile([P, 512], bf16, tag="te", bufs=2)
                        nc.vector.tensor_tensor(te[:], m1[:], wps[:, 0, :], OP.add)
                        to = cvsb.tile([P, 512], bf16, tag="to", bufs=2)
                        nc.vector.tensor_tensor(to[:], m1[:], wps[:, 2, :], OP.subtract)
                        nc.vector.tensor_tensor(dst_even, te[:], wps[:, 2, :], OP.add)
                        nc.vector.tensor_tensor(dst_odd, to[:], wps[:, 3, :], OP.subtract)

                    for m in range(CH):
                        # f5: outer taps (shifts +-2) direct into cps5
                        cps5 = cvps.tile([P, S], f32, tag="cps", bufs=2)
                        for ti, (tp, shift) in enumerate(((0, -2), (4, 2))):
                            cwblk = cvsb.tile([P, CH, P], bf16, tag="cw", bufs=4)
                            nc.sync.dma_start(cwblk[:], cw_d[tp, m])
                            for c in range(CH):
                                for n in range(NH):
                                    sl = slice(n * 512, (n + 1) * 512)
                                    u0 = 2 + shift + n * 512
                                    nc.tensor.matmul(cps5[:, sl], cwblk[:, c, :],
                                                     g[:, c, u0:u0 + 512],
                                                     start=(ti == 0 and c == 0),
                                                     stop=(ti == 1 and c == CH - 1))
                        # f5 inner 3 taps via Winograd -> w5 (interleaved)
                        w5 = cvsb.tile([P, S], bf16, tag="w5", bufs=2)
                        w5v = w5[:].rearrange("p (t e) -> p e t", e=2)
                        wps5 = cvps.tile([P, 4, 512], f32, tag="wps", bufs=1)
                        wino(wps5, m, 4, w5v[:, 0, :], w5v[:, 1, :])
                        # f1 direct (PE busy while f5 combine drains wps)
                        cps1 = cvps.tile([P, S], f32, tag="cps", bufs=2)
                        cwblk = cvsb.tile([P, CH, P], bf16, tag="cw", bufs=4)
                        nc.sync.dma_start(cwblk[:], cw_d[8, m])
                        for c in range(CH):
                            for n in range(NH):
                                sl = slice(n * 512, (n + 1) * 512)
                                nc.tensor.matmul(cps1[:, sl], cwblk[:, c, :],
                                                 g[:, c, 2 + n * 512:2 + n * 512 + 512],
                                                 start=(c == 0), stop=(c == CH - 1))
                        sc1 = cvsb.tile([P, S], bf16, tag="scr2", bufs=2)
                        nc.scalar.activation(sc1[:], cps1[:], AF.Relu, bias=cb_sb[:, 2, m:m + 1])
                        # f3 via Winograd -> sc3 (strided relu)
                        sc3 = cvsb.tile([P, S], bf16, tag="scr1", bufs=2)
                        ye = cvsb.tile([P, 512], bf16, tag="ye", bufs=2)
                        yo = cvsb.tile([P, 512], bf16, tag="yo", bufs=2)
                        wps3 = cvps.tile([P, 4, 512], f32, tag="wps", bufs=1)
                        wino(wps3, m, 0, ye[:], yo[:])
                        sc3v = sc3[:].rearrange("p (t e) -> p e t", e=2)
                        nc.scalar.activation(sc3v[:, 0, :], ye[:], AF.Relu,
                                             bias=cb_sb[:, 1, m:m + 1])
                        nc.scalar.activation(sc3v[:, 1, :], yo[:], AF.Relu,
                                             bias=cb_sb[:, 1, m:m + 1])
                        # f5 = relu(cps5 + w5 + bias)
                        y5 = cvsb.tile([P, S], bf16, tag="y5", bufs=2)
                        nc.vector.tensor_tensor(y5[:], cps5[:], w5[:], OP.add)
                        sc5 = cvsb.tile([P, S], bf16, tag="scr0", bufs=2)
                        nc.scalar.activation(sc5[:], y5[:], AF.Relu, bias=cb_sb[:, 0, m:m + 1])
                        t1 = cvsb.tile([P, S], bf16, tag="cmb", bufs=2)
                        nc.vector.tensor_tensor(t1[:], sc5[:], sc3[:], OP.add)
                        t2 = cvsb.tile([P, S], bf16, tag="cmb2", bufs=2)
                        nc.vector.tensor_tensor(t2[:], t1[:], sc1[:], OP.add)
                        nc.vector.tensor_tensor(x[:, m, :], x[:, m, :], t2[:], OP.add)

            # ---------------- final LN + writeback ----------------
            emit_ln(None, 0)

    nc.compile()
    return nc, names


_BUILT = None


def _get_built():
    global _BUILT
    if _BUILT is None:
        _BUILT = _build()
    return _BUILT


def _pack_fp8(w):
    # w: [D, D] contraction-major (w[k, o]) -> [CH_m, P, KP, 2, P] fp8 with
    # pk[m, p, kp, j, n] = w[(2*kp+j)*128 + p, 128*m + n] * WSCALE
    pk = (w * WSCALE).reshape(KP, 2, P, CH, P).transpose(3, 2, 0, 1, 4)
    return np.ascontiguousarray(pk).astype(ml_dtypes.float8_e4m3)


def _pack_lhsT(w):
    # w: [D, D] contraction-major -> [CH_m, P, CH_k, P] with pk[m,p,k,n] = w[128k+p, 128m+n]
    return np.ascontiguousarray(w.reshape(CH, P, CH, P).transpose(2, 1, 0, 3))


def _pack_bias(b):
    return np.ascontiguousarray(b.reshape(CH, P).T)


def _prep(inputs):
    f = lambda k: np.asarray(inputs[k], np.float32)
    a1, b1 = f('ln1_a'), f('ln1_b')
    a2, b2 = f('ln2_a'), f('ln2_b')
    wq, wk, wv, wo = f('wq'), f('wk'), f('wv'), f('wo')
    bq, bk, bv, bo = f('bq'), f('bk'), f('bv'), f('bo')

    d = {}
    d['wq'] = _pack_fp8((a1[:, None] * wq) / 8.0)
    d['bq'] = _pack_bias((bq + b1 @ wq) / 8.0)
    d['wk'] = _pack_fp8(a1[:, None] * wk)
    d['bk'] = _pack_bias(bk + b1 @ wk)
    d['wv'] = np.ascontiguousarray(
        (a1[:, None] * wv * WSCALE).reshape(CH, P, S)).astype(ml_dtypes.float8_e4m3)
    d['bv'] = (bv + b1 @ wv).reshape(1, S)
    d['wo'] = _pack_fp8(wo)
    d['bo'] = _pack_bias(bo)

    cw = np.empty((N_TAPS, CH, P, CH, P), ml_dtypes.bfloat16)
    uw = np.empty((8, CH, P, CH, P), ml_dtypes.bfloat16)
    cb = np.empty((P, 3, CH), np.float32)
    for bi, fs in enumerate((5, 3, 1)):
        i = 3 - bi   # conv_w1 is the 1-tap filter, conv_w3 the 5-tap one
        W = f(f'conv_w{i}')        # [oc, ic, f]
        b = f(f'conv_b{i}')
        g_, beta = f(f'bn_g{i}'), f(f'bn_b{i}')
        m, v = f(f'bn_m{i}'), f(f'bn_v{i}')
        s = g_ / np.sqrt(v + EPS_BN)
        Wf = W * s[:, None, None] * a2[None, :, None] / 3.0
        bias = ((b + W.sum(axis=2) @ b2 - m) * s + beta) / 3.0
        cb[:, bi, :] = _pack_bias(bias)
        if fs in (3, 5):
            j0 = 0 if fs == 3 else 1      # inner 3 taps (shifts -1,0,1)
            base = 0 if fs == 3 else 4
            W0, W1, W2 = Wf[:, :, j0], Wf[:, :, j0 + 1], Wf[:, :, j0 + 2]
            U = (W0, (W0 + W1 + W2) * 0.5, (W0 - W1 + W2) * 0.5, W2)
            for i in range(4):
                uw[base + i] = _pack_lhsT(np.ascontiguousarray(U[i].T)).astype(ml_dtypes.bfloat16)
        for j, (tp, _) in enumerate(BRANCH_TAPS[bi]):
            cw[tp] = _pack_lhsT(np.ascontiguousarray(Wf[:, :, j].T)).astype(ml_dtypes.bfloat16)
    d['cw'] = cw
    d['uw'] = uw
    d['cb'] = cb
    return d


def kernel(**inputs):
    nc, names = _get_built()
    shared = _prep(inputs)
    x = np.asarray(inputs['x'], np.float32)
    in_maps = []
    for b in range(N_CORES):
        m = {names[k]: v for k, v in shared.items()}
        m[names['xt']] = np.ascontiguousarray(x[b].T).astype(ml_dtypes.bfloat16)
        in_maps.append(m)
    res = run_bass_kernel_spmd(nc, in_maps, core_ids=list(range(N_CORES)))
    af = np.asarray(inputs['lnf_a'], np.float32)
    bf = np.asarray(inputs['lnf_b'], np.float32)
    out = np.empty((N_CORES, S, D), np.float32)
    for b in range(N_CORES):
        yt = np.asarray(res.results[b][names['yt']], np.float32)
        out[b] = yt.T * af[None, :] + bf[None, :]
    return out


# revision 25
# speedup vs baseline: 1.0112x; 1.0112x over previous
"""Trainium2 Bass kernel for the shared-weight transformer encoder with a
Conv1d-ensemble FFN (nn_MCAT_23630910062939).

Sharding: data-parallel over batch — each of the 8 NeuronCores computes one
full batch element; no collectives.  The residual stream lives on-chip in
feature-major layout x^T [D, S] (bf16); host transposes input/output.

Precision plan (validated against a numpy error model, all-batch max rel
6.4e-3 vs the 2e-2 gate): the conv ensemble (60% of the MACs) runs in bf16 —
fp8 there blows the error budget.  The attention-side projections (Q, K, V,
output) run in fp8e4m3 DoubleRow mode (2 rows/cycle); their weights are
pre-scaled by 64 on the host and descaled in the PSUM-drain activation.
DoubleRow outputs must land on PSUM partitions 0-63 (s3d3 ISA rule), so
projections use two 64-row psum tiles per 128-feature chunk.
"""
import sys, os
sys.path.insert(0, '/opt/trn_rl_repo')
import numpy as np
import ml_dtypes

from contextlib import ExitStack
import concourse.bass as bass
import concourse.mybir as mybir
import concourse.tile as tile
from concourse import bacc, library_config
from concourse.bass_utils import run_bass_kernel_spmd

P = 128
D = 1024
S = 1024
H = 16
DK = 64
CH = D // P          # 8 feature chunks
KP = CH // 2         # 4 DoubleRow k-tile pairs
NH = 2               # 512-wide halves of the token axis
NL = 2               # shared layer applied twice
N_CORES = 8
EPS_LN = 1e-6
EPS_BN = 1e-5
WSCALE = 64.0        # fp8 weight pre-scale (w sigma 0.02 -> 1.3)

f32 = mybir.dt.float32
f32r = mybir.dt.float32r
bf16 = mybir.dt.bfloat16
fp8 = mybir.dt.float8e4
AF = mybir.ActivationFunctionType
OP = mybir.AluOpType
PM = mybir.MatmulPerfMode

# conv taps in pack order: (pack_idx, branch, shift)
# branch 0: filter 5 (shifts -2..2), branch 1: filter 3 (-1..1), branch 2: filter 1 (0)
BRANCH_TAPS = [
    [(0, -2), (1, -1), (2, 0), (3, 1), (4, 2)],
    [(5, -1), (6, 0), (7, 1)],
    [(8, 0)],
]
N_TAPS = 9


def _build():
    nc = bacc.Bacc(None, target_bir_lowering=False)
    names = {}

    def reg(t, key):
        names[key] = t.name
        return t

    with tile.TileContext(nc) as tc, ExitStack() as stack:
        with tc.tile_pool(name="dram", bufs=1, space="DRAM") as dram:
            xt_d = reg(dram.tile([D, S], bf16, kind="ExternalInput", name="xt"), "xt")
            # fp8 DoubleRow weight blocks: [m-chunk, partition(k%128), kpair, j, out(128)]
            wq_d = reg(dram.tile([CH, P, KP, 2, P], fp8, kind="ExternalInput", name="wq"), "wq")
            wk_d = reg(dram.tile([CH, P, KP, 2, P], fp8, kind="ExternalInput", name="wk"), "wk")
            wo_d = reg(dram.tile([CH, P, KP, 2, P], fp8, kind="ExternalInput", name="wo"), "wo")
            wv_d = reg(dram.tile([CH, P, S], fp8, kind="ExternalInput", name="wv"), "wv")
            cw_d = reg(dram.tile([N_TAPS, CH, P, CH, P], bf16, kind="ExternalInput", name="cw"), "cw")
            uw_d = reg(dram.tile([8, CH, P, CH, P], bf16, kind="ExternalInput", name="uw"), "uw")
            bq_d = reg(dram.tile([P, CH], f32, kind="ExternalInput", name="bq"), "bq")
            bk_d = reg(dram.tile([P, CH], f32, kind="ExternalInput", name="bk"), "bk")
            bo_d = reg(dram.tile([P, CH], f32, kind="ExternalInput", name="bo"), "bo")
            bv_d = reg(dram.tile([1, S], f32, kind="ExternalInput", name="bv"), "bv")
            cb_d = reg(dram.tile([P, 3, CH], f32, kind="ExternalInput", name="cb"), "cb")
            yt_d = reg(dram.tile([D, S], bf16, kind="ExternalOutput", name="yt"), "yt")

            nc.gpsimd.load_library(library_config.attn)

            const = stack.enter_context(tc.tile_pool(name="const", bufs=1))
            ones32 = const.tile([P, 1], f32)
            nc.vector.memset(ones32[:], 1.0 / D)
            ones_b = const.tile([P, 1], bf16)
            nc.vector.tensor_copy(ones_b[:], ones32[:])
            bq_sb = const.tile([P, CH], f32)
            bk_sb = const.tile([P, CH], f32)
            bo_sb = const.tile([P, CH], f32)
            cb_sb = const.tile([P, 3, CH], f32)
            nc.sync.dma_start(bq_sb[:], bq_d[:])
            nc.sync.dma_start(bk_sb[:], bk_d[:])
            nc.sync.dma_start(bo_sb[:], bo_d[:])
            nc.sync.dma_start(cb_sb[:], cb_d[:])
            bv_row = const.tile([1, S], f32)
            nc.sync.dma_start(bv_row[:], bv_d[:])
            bv_b = const.tile([P, S], f32)
            nc.gpsimd.partition_broadcast(bv_b[:], bv_row[:])

            glob = stack.enter_context(tc.tile_pool(name="glob", bufs=1))
            x = glob.tile([P, CH, S], bf16, tag="x")          # residual x^T
            act8 = glob.tile([P, CH, S], fp8, tag="act8")     # LN1 output h (fp8)
            g = glob.tile([P, CH, S + 4], bf16, tag="g")      # LN2 output (padded)
            vw8 = glob.tile([P, CH, S], fp8, tag="vw8")       # V weights (resident)
            z = glob.tile([P, CH, 4, 512], bf16, tag="z")     # F(2,3) input transform
            for c in range(CH):
                nc.sync.dma_start(x[:, c, :], xt_d[c * P:(c + 1) * P, :])
            for c in range(CH):
                nc.sync.dma_start(vw8[:, c, :], wv_d[c])
            z16 = const.tile([P, 16], bf16)
            nc.vector.memset(z16[:], 0.0)
            zv = z16[:].rearrange("p (c e) -> p c e", e=2)
            nc.vector.tensor_copy(g[:, :, 0:2], zv)
            nc.vector.tensor_copy(g[:, :, S + 2:S + 4], zv)

            def emit_ln(dst, dst_off, sq_on_act=True):
                """LayerNorm over the feature (partition) axis of x.
                dst: act8 (fp8, attention input), g (bf16, conv input, +2 col
                offset), or None (final LN -> DMA f32 to yt_d)."""
                with tc.tile_pool(name="lnps", bufs=1, space="PSUM") as lnps, \
                     tc.tile_pool(name="lnsb", bufs=1) as lnsb:
                    mean_ps = lnps.tile([1, S], f32, tag="mean")
                    msq_ps = lnps.tile([1, S], f32, tag="msq")
                    for c in range(CH):
                        sq = lnsb.tile([P, S], bf16, tag="sq", bufs=2)
                        if sq_on_act:
                            nc.scalar.activation(sq[:], x[:, c, :], AF.Square)
                        else:
                            nc.vector.tensor_mul(sq[:], x[:, c, :], x[:, c, :])
                        for n in range(NH):
                            sl = slice(n * 512, (n + 1) * 512)
                            nc.tensor.matmul(mean_ps[:, sl], ones_b[:], x[:, c, sl],
                                             start=(c == 0), stop=(c == CH - 1))
                            nc.tensor.matmul(msq_ps[:, sl], ones_b[:], sq[:, sl],
                                             start=(c == 0), stop=(c == CH - 1))
                    lp = nc.allow_low_precision(reason="LN stats rows in bf16; 0.4% std err ok")
                    lp.__enter__()
                    mean_sb = lnsb.tile([1, S], bf16, tag="mrow")
                    nc.vector.tensor_copy(mean_sb[:], mean_ps[:])
                    m2 = lnsb.tile([1, S], bf16, tag="m2")
                    nc.vector.tensor_mul(m2[:], mean_sb[:], mean_sb[:])
                    var0 = lnsb.tile([1, S], bf16, tag="var0")
                    nc.vector.tensor_tensor(var0[:], msq_ps[:], m2[:], OP.subtract)
                    # unbiased std = sqrt(var0 * D/(D-1)); eps ~ 1e-6 is
                    # immaterial at our error scale.  Sqrt shares an ACT table
                    # set with Square/Identity/Relu (no table swap), and
                    # reciprocal runs on DVE.
                    stdr = lnsb.tile([1, S], bf16, tag="stdr")
                    nc.scalar.activation(stdr[:], var0[:], AF.Sqrt, scale=float(D / (D - 1.0)))
                    inv_rb = lnsb.tile([1, S], bf16, tag="invrb")
                    nc.vector.reciprocal(inv_rb[:], stdr[:])
                    minv_rb = lnsb.tile([1, S], bf16, tag="minvrb")
                    nc.vector.tensor_mul(minv_rb[:], mean_sb[:], inv_rb[:])
                    inv_b = lnsb.tile([P, S], bf16, tag="invb")
                    minv_b = lnsb.tile([P, S], bf16, tag="minvb")
                    nc.gpsimd.partition_broadcast(inv_b[:], inv_rb[:])
                    nc.gpsimd.partition_broadcast(minv_b[:], minv_rb[:])
                    lp.__exit__(None, None, None)
                    for c in range(CH):
                        t = lnsb.tile([P, S], bf16, tag="lnapp", bufs=2)
                        nc.vector.tensor_mul(t[:], x[:, c, :], inv_b[:])
                        if dst is not None:
                            nc.vector.tensor_tensor(dst[:, c, dst_off:dst_off + S],
                                                    t[:], minv_b[:], OP.subtract)
                        else:
                            o = lnsb.tile([P, S], bf16, tag="lnout", bufs=2)
                            nc.vector.tensor_tensor(o[:], t[:], minv_b[:], OP.subtract)
                            nc.sync.dma_start(yt_d[c * P:(c + 1) * P, :], o[:])

            for layer in range(NL):
                # ---------------- LN1 -> h (act8, fp8) ----------------
                emit_ln(act8, 0)

                with tc.tile_pool(name="attnbuf", bufs=1) as ab:
                    qt = ab.tile([P, CH, S], bf16, tag="qt")
                    kt = ab.tile([P, CH, S], bf16, tag="kt")
                    vt = ab.tile([P, CH, H * 65], bf16, tag="vt")
                    ot = ab.tile([P, CH, S], fp8, tag="ot")

                    # ---------------- Q,K projections (fp8 DoubleRow) --------
                    # Q/K interleaved per feature-chunk m so attention head
                    # pair 2m,2m+1 unblocks right after chunk m; Q drains on
                    # ACT, K on DVE (tensor_scalar) to split the drain load.
                    with tc.tile_pool(name="qkps", bufs=1, space="PSUM") as qkps, \
                         tc.tile_pool(name="qksb", bufs=1) as qksb:
                        for m in range(CH):
                            for (w_d, bsb, dst, on_act) in ((wq_d, bq_sb, qt, True),
                                                            (wk_d, bk_sb, kt, False)):
                                wblk = qksb.tile([P, KP, 2, P], fp8, tag="wproj", bufs=4)
                                nc.sync.dma_start(wblk[:], w_d[m])
                                for mb in range(2):
                                    msl = slice(mb * 64, (mb + 1) * 64)
                                    ps = qkps.tile([64, S], f32, tag=f"pps{mb}", bufs=1)
                                    for kp in range(KP):
                                        for n in range(NH):
                                            nc.tensor.matmul(
                                                ps[:, n * 512:(n + 1) * 512],
                                                wblk[:, kp, :, msl],
                                                act8[:, 2 * kp:2 * kp + 2, n * 512:(n + 1) * 512],
                                                start=(kp == 0), stop=(kp == KP - 1),
                                                perf_mode=PM.DoubleRow)
                                    if on_act:
                                        nc.scalar.activation(dst[msl, m, :], ps[:], AF.Identity,
                                                             bias=bsb[msl, m:m + 1], scale=1.0 / WSCALE)
                                    else:
                                        nc.vector.tensor_scalar(dst[msl, m, :], ps[:],
                                                                1.0 / WSCALE, bsb[msl, m:m + 1],
                                                                op0=OP.mult, op1=OP.add)

                        # ---------- V projection (fp8 DoubleRow, token-major)
                        nc.vector.memset(vt[:].rearrange("p c (h e) -> p c h e", e=65)[:, :, :, 64:65], 1.0)
                        for tb in range(2 * CH):
                            pv = qkps.tile([64, S], f32, tag="pv", bufs=2)
                            for kp in range(KP):
                                for n in range(NH):
                                    nc.tensor.matmul(
                                        pv[:, n * 512:(n + 1) * 512],
                                        act8[:, 2 * kp:2 * kp + 2, tb * 64:(tb + 1) * 64],
                                        vw8[:, 2 * kp:2 * kp + 2, n * 512:(n + 1) * 512],
                                        start=(kp == 0), stop=(kp == KP - 1),
                                        perf_mode=PM.DoubleRow)
                            p0 = (tb % 2) * 64
                            dstv = vt[p0:p0 + 64, tb // 2, :].rearrange("p (h e) -> p h e", e=65)[:, :, 0:64]
                            src = pv[:].rearrange("p (h e) -> p h e", e=64)
                            bvv = bv_b[0:64, :].rearrange("p (h e) -> p h e", e=64)
                            nc.vector.scalar_tensor_tensor(dstv, src, 1.0 / WSCALE, bvv,
                                                           op0=OP.mult, op1=OP.add)

                    # ---------------- attention per head ----------------
                    with tc.tile_pool(name="atps", bufs=1, space="PSUM") as atps, \
                         tc.tile_pool(name="atsb", bufs=1) as atsb:
                        for h in range(H):
                            hp, off = h // 2, 64 * (h % 2)
                            pexp = atsb.tile([P, CH, S], bf16, tag="pexp", bufs=2)
                            for mk in range(CH):
                                scps = atps.tile([P, S], f32, tag="scps", bufs=2)
                                for n in range(NH):
                                    nc.tensor.matmul(scps[:, n * 512:(n + 1) * 512],
                                                     kt[off:off + 64, hp, mk * P:(mk + 1) * P],
                                                     qt[off:off + 64, hp, n * 512:(n + 1) * 512],
                                                     start=True, stop=True)
                                nc.scalar.activation(pexp[:, mk, :], scps[:], AF.Exp)
                            ops = atps.tile([65, S], f32, tag="ops", bufs=2)
                            for c in range(CH):
                                for n in range(NH):
                                    sl = slice(n * 512, (n + 1) * 512)
                                    nc.tensor.matmul(ops[:, sl], vt[:, c, 65 * h:65 * h + 65],
                                                     pexp[:, c, sl],
                                                     start=(c == 0), stop=(c == CH - 1))
                            rrow = atsb.tile([1, S], f32, tag="rrow", bufs=2)
                            nc.vector.reciprocal(rrow[:], ops[64:65, :])
                            rb = atsb.tile([64, S], f32, tag="rb", bufs=2)
                            nc.gpsimd.partition_broadcast(rb[:], rrow[:])
                            nc.vector.tensor_tensor(ot[off:off + 64, hp, :], ops[0:64, :], rb[:], OP.mult)

                    # ---------------- output projection + residual ----------
                    with tc.tile_pool(name="wops", bufs=1, space="PSUM") as wops, \
                         tc.tile_pool(name="wosb", bufs=1) as wosb:
                        for m in range(CH):
                            woblk = wosb.tile([P, KP, 2, P], fp8, tag="wo", bufs=3)
                            nc.sync.dma_start(woblk[:], wo_d[m])
                            att = wosb.tile([P, S], bf16, tag="att", bufs=2)
                            for mb in range(2):
                                msl = slice(mb * 64, (mb + 1) * 64)
                                ps = wops.tile([64, S], f32, tag=f"ops{mb}", bufs=2)
                                for kp in range(KP):
                                    for n in range(NH):
                                        nc.tensor.matmul(
                                            ps[:, n * 512:(n + 1) * 512],
                                            woblk[:, kp, :, msl],
                                            ot[:, 2 * kp:2 * kp + 2, n * 512:(n + 1) * 512],
                                            start=(kp == 0), stop=(kp == KP - 1),
                                            perf_mode=PM.DoubleRow)
                                nc.scalar.activation(att[msl, :], ps[:], AF.Identity,
                                                     bias=bo_sb[msl, m:m + 1], scale=1.0 / WSCALE)
                            nc.vector.tensor_tensor(x[:, m, :], x[:, m, :], att[:], OP.add)

                # ---------------- LN2 -> g (bf16, padded) ----------------
                emit_ln(g, 2, sq_on_act=False)

                # F(2,3) input transform for the 3-tap branch: for output pair
                # (2u, 2u+1): d = g(2u-1 .. 2u+2); z0 = d0-d2, z1 = d1+d2,
                # z2 = d2-d1, z3 = d1-d3.  view(a) = g cols a+2t (t in 0..511).
                for c in range(CH):
                    view = lambda a: g[:, c, a:a + 1024].rearrange(
                        "p (t e) -> p e t", e=2)[:, 0, :]
                    nc.vector.tensor_tensor(z[:, c, 0, :], view(1), view(3), OP.subtract)
                    nc.vector.tensor_tensor(z[:, c, 1, :], view(2), view(3), OP.add)
                    nc.vector.tensor_tensor(z[:, c, 2, :], view(3), view(2), OP.subtract)
                    nc.vector.tensor_tensor(z[:, c, 3, :], view(2), view(4), OP.subtract)

                # ---------------- conv ensemble FFN (bf16) -------
                with tc.tile_pool(name="cvps", bufs=1, space="PSUM") as cvps, \
                     tc.tile_pool(name="cvsb", bufs=1) as cvsb:
                    def wino(wps, m, base, dst_even, dst_odd):
                        # phases m_i = U_i^T z_i; y_even = m0+m1+m2, y_odd = m1-m2-m3
                        for i in range(4):
                            uwblk = cvsb.tile([P, CH, P], bf16, tag="uw", bufs=6)
                            nc.sync.dma_start(uwblk[:], uw_d[base + i, m])
                            for c in range(CH):
                                nc.tensor.matmul(wps[:, i, :], uwblk[:, c, :],
                                                 z[:, c, i, :],
                                                 start=(c == 0), stop=(c == CH - 1))
                        # DVE may read only one PSUM operand per op: stage m1
                        m1 = cvsb.tile([P, 512], bf16, tag="m1s", bufs=2)
                        nc.vector.tensor_copy(m1[:], wps[:, 1, :])
                        te = cvsb.tile([P, 512], bf16, tag="te", bufs=2)
                        nc.vector.tensor_tensor(te[:], m1[:], wps[:, 0, :], OP.add)
                        to = cvsb.tile([P, 512], bf16, tag="to", bufs=2)
                        nc.vector.tensor_tensor(to[:], m1[:], wps[:, 2, :], OP.subtract)
                        nc.vector.tensor_tensor(dst_even, te[:], wps[:, 2, :], OP.add)
                        nc.vector.tensor_tensor(dst_odd, to[:], wps[:, 3, :], OP.subtract)

                    for m in range(CH):
                        # f5: outer taps (shifts +-2) direct into cps5
                        cps5 = cvps.tile([P, S], f32, tag="cps", bufs=2)
                        for ti, (tp, shift) in enumerate(((0, -2), (4, 2))):
                            cwblk = cvsb.tile([P, CH, P], bf16, tag="cw", bufs=4)
                            nc.sync.dma_start(cwblk[:], cw_d[tp, m])
                            for c in range(CH):
                                for n in range(NH):
                                    sl = slice(n * 512, (n + 1) * 512)
                                    u0 = 2 + shift + n * 512
                                    nc.tensor.matmul(cps5[:, sl], cwblk[:, c, :],
                                                     g[:, c, u0:u0 + 512],
                                                     start=(ti == 0 and c == 0),
                                                     stop=(ti == 1 and c == CH - 1))
                        # f5 inner 3 taps via Winograd -> w5 (interleaved)
                        w5 = cvsb.tile([P, S], bf16, tag="w5", bufs=2)
                        w5v = w5[:].rearrange("p (t e) -> p e t", e=2)
                        wps5 = cvps.tile([P, 4, 512], f32, tag="wps", bufs=1)
                        wino(wps5, m, 4, w5v[:, 0, :], w5v[:, 1, :])
                        # f1 direct (PE busy while f5 combine drains wps)
                        cps1 = cvps.tile([P, S], f32, tag="cps", bufs=2)
                        cwblk = cvsb.tile([P, CH, P], bf16, tag="cw", bufs=4)
                        nc.sync.dma_start(cwblk[:], cw_d[8, m])
                        for c in range(CH):
                            for n in range(NH):
                                sl = slice(n * 512, (n + 1) * 512)
                                nc.tensor.matmul(cps1[:, sl], cwblk[:, c, :],
                                                 g[:, c, 2 + n * 512:2 + n * 512 + 512],
                                                 start=(c == 0), stop=(c == CH - 1))
                        sc1 = cvsb.tile([P, S], bf16, tag="scr2", bufs=2)
                        nc.scalar.activation(sc1[:], cps1[:], AF.Relu, bias=cb_sb[:, 2, m:m + 1])
                        # f3 via Winograd -> sc3 (strided relu)
                        sc3 = cvsb.tile([P, S], bf16, tag="scr1", bufs=2)
                        ye = cvsb.tile([P, 512], bf16, tag="ye", bufs=2)
                        yo = cvsb.tile([P, 512], bf16, tag="yo", bufs=2)
                        wps3 = cvps.tile([P, 4, 512], f32, tag="wps", bufs=1)
                        wino(wps3, m, 0, ye[:], yo[:])
                        sc3v = sc3[:].rearrange("p (t e) -> p e t", e=2)
                        nc.scalar.activation(sc3v[:, 0, :], ye[:], AF.Relu,
                                             bias=cb_sb[:, 1, m:m + 1])
                        nc.scalar.activation(sc3v[:, 1, :], yo[:], AF.Relu,
                                             bias=cb_sb[:, 1, m:m + 1])
                        # f5 = relu(cps5 + w5 + bias)
                        y5 = cvsb.tile([P, S], bf16, tag="y5", bufs=2)
                        nc.vector.tensor_tensor(y5[:], cps5[:], w5[:], OP.add)
                        sc5 = cvsb.tile([P, S], bf16, tag="scr0", bufs=2)
                        nc.scalar.activation(sc5[:], y5[:], AF.Relu, bias=cb_sb[:, 0, m:m + 1])
                        t1 = cvsb.tile([P, S], bf16, tag="cmb", bufs=2)
                        nc.vector.tensor_tensor(t1[:], sc5[:], sc3[:], OP.add)
                        t2 = cvsb.tile([P, S], bf16, tag="cmb2", bufs=2)
                        nc.vector.tensor_tensor(t2[:], t1[:], sc1[:], OP.add)
                        nc.vector.tensor_tensor(x[:, m, :], x[:, m, :], t2[:], OP.add)

            # ---------------- final LN + writeback ----------------
            emit_ln(None, 0)

    nc.compile()
    return nc, names


_BUILT = None


def _get_built():
    global _BUILT
    if _BUILT is None:
        _BUILT = _build()
    return _BUILT


def _pack_fp8(w):
    # w: [D, D] contraction-major (w[k, o]) -> [CH_m, P, KP, 2, P] fp8 with
    # pk[m, p, kp, j, n] = w[(2*kp+j)*128 + p, 128*m + n] * WSCALE
    pk = (w * WSCALE).reshape(KP, 2, P, CH, P).transpose(3, 2, 0, 1, 4)
    return np.ascontiguousarray(pk).astype(ml_dtypes.float8_e4m3)


def _pack_lhsT(w):
    # w: [D, D] contraction-major -> [CH_m, P, CH_k, P] with pk[m,p,k,n] = w[128k+p, 128m+n]
    return np.ascontiguousarray(w.reshape(CH, P, CH, P).transpose(2, 1, 0, 3))


def _pack_bias(b):
    return np.ascontiguousarray(b.reshape(CH, P).T)


def _prep(inputs):
    f = lambda k: np.asarray(inputs[k], np.float32)
    a1, b1 = f('ln1_a'), f('ln1_b')
    a2, b2 = f('ln2_a'), f('ln2_b')
    wq, wk, wv, wo = f('wq'), f('wk'), f('wv'), f('wo')
    bq, bk, bv, bo = f('bq'), f('bk'), f('bv'), f('bo')

    d = {}
    d['wq'] = _pack_fp8((a1[:, None] * wq) / 8.0)
    d['bq'] = _pack_bias((bq + b1 @ wq) / 8.0)
    d['wk'] = _pack_fp8(a1[:, None] * wk)
    d['bk'] = _pack_bias(bk + b1 @ wk)
    d['wv'] = np.ascontiguousarray(
        (a1[:, None] * wv * WSCALE).reshape(CH, P, S)).astype(ml_dtypes.float8_e4m3)
    d['bv'] = (bv + b1 @ wv).reshape(1, S)
    d['wo'] = _pack_fp8(wo)
    d['bo'] = _pack_bias(bo)

    cw = np.empty((N_TAPS, CH, P, CH, P), ml_dtypes.bfloat16)
    uw = np.empty((8, CH, P, CH, P), ml_dtypes.bfloat16)
    cb = np.empty((P, 3, CH), np.float32)
    for bi, fs in enumerate((5, 3, 1)):
        i = 3 - bi   # conv_w1 is the 1-tap filter, conv_w3 the 5-tap one
        W = f(f'conv_w{i}')        # [oc, ic, f]
        b = f(f'conv_b{i}')
        g_, beta = f(f'bn_g{i}'), f(f'bn_b{i}')
        m, v = f(f'bn_m{i}'), f(f'bn_v{i}')
        s = g_ / np.sqrt(v + EPS_BN)
        Wf = W * s[:, None, None] * a2[None, :, None] / 3.0
        bias = ((b + W.sum(axis=2) @ b2 - m) * s + beta) / 3.0
        cb[:, bi, :] = _pack_bias(bias)
        if fs in (3, 5):
            j0 = 0 if fs == 3 else 1      # inner 3 taps (shifts -1,0,1)
            base = 0 if fs == 3 else 4
            W0, W1, W2 = Wf[:, :, j0], Wf[:, :, j0 + 1], Wf[:, :, j0 + 2]
            U = (W0, (W0 + W1 + W2) * 0.5, (W0 - W1 + W2) * 0.5, W2)
            for i in range(4):
                uw[base + i] = _pack_lhsT(np.ascontiguousarray(U[i].T)).astype(ml_dtypes.bfloat16)
        for j, (tp, _) in enumerate(BRANCH_TAPS[bi]):
            cw[tp] = _pack_lhsT(np.ascontiguousarray(Wf[:, :, j].T)).astype(ml_dtypes.bfloat16)
    d['cw'] = cw
    d['uw'] = uw
    d['cb'] = cb
    return d


def kernel(**inputs):
    nc, names = _get_built()
    shared = _prep(inputs)
    x = np.asarray(inputs['x'], np.float32)
    in_maps = []
    for b in range(N_CORES):
        m = {names[k]: v for k, v in shared.items()}
        m[names['xt']] = np.ascontiguousarray(x[b].T).astype(ml_dtypes.bfloat16)
        in_maps.append(m)
    res = run_bass_kernel_spmd(nc, in_maps, core_ids=list(range(N_CORES)))
    af = np.asarray(inputs['lnf_a'], np.float32)
    bf = np.asarray(inputs['lnf_b'], np.float32)
    out = np.empty((N_CORES, S, D), np.float32)
    for b in range(N_CORES):
        yt = np.asarray(res.results[b][names['yt']], np.float32)
        out[b] = yt.T * af[None, :] + bf[None, :]
    return out


# revision 27
# speedup vs baseline: 1.0177x; 1.0064x over previous
"""Trainium2 Bass kernel for the shared-weight transformer encoder with a
Conv1d-ensemble FFN (nn_MCAT_23630910062939).

Sharding: data-parallel over batch — each of the 8 NeuronCores computes one
full batch element; no collectives.  The residual stream lives on-chip in
feature-major layout x^T [D, S] (bf16); host transposes input/output.

Precision plan (validated against a numpy error model, all-batch max rel
6.4e-3 vs the 2e-2 gate): the conv ensemble (60% of the MACs) runs in bf16 —
fp8 there blows the error budget.  The attention-side projections (Q, K, V,
output) run in fp8e4m3 DoubleRow mode (2 rows/cycle); their weights are
pre-scaled by 64 on the host and descaled in the PSUM-drain activation.
DoubleRow outputs must land on PSUM partitions 0-63 (s3d3 ISA rule), so
projections use two 64-row psum tiles per 128-feature chunk.
"""
import sys, os
sys.path.insert(0, '/opt/trn_rl_repo')
import numpy as np
import ml_dtypes

from contextlib import ExitStack
import concourse.bass as bass
import concourse.mybir as mybir
import concourse.tile as tile
from concourse import bacc, library_config
from concourse.bass_utils import run_bass_kernel_spmd

P = 128
D = 1024
S = 1024
H = 16
DK = 64
CH = D // P          # 8 feature chunks
KP = CH // 2         # 4 DoubleRow k-tile pairs
NH = 2               # 512-wide halves of the token axis
NL = 2               # shared layer applied twice
N_CORES = 8
EPS_LN = 1e-6
EPS_BN = 1e-5
WSCALE = 64.0        # fp8 weight pre-scale (w sigma 0.02 -> 1.3)

f32 = mybir.dt.float32
f32r = mybir.dt.float32r
bf16 = mybir.dt.bfloat16
fp8 = mybir.dt.float8e4
AF = mybir.ActivationFunctionType
OP = mybir.AluOpType
PM = mybir.MatmulPerfMode

# conv taps in pack order: (pack_idx, branch, shift)
# branch 0: filter 5 (shifts -2..2), branch 1: filter 3 (-1..1), branch 2: filter 1 (0)
BRANCH_TAPS = [
    [(0, -2), (1, -1), (2, 0), (3, 1), (4, 2)],
    [(5, -1), (6, 0), (7, 1)],
    [(8, 0)],
]
N_TAPS = 9


def _build():
    nc = bacc.Bacc(None, target_bir_lowering=False)
    names = {}

    def reg(t, key):
        names[key] = t.name
        return t

    with tile.TileContext(nc) as tc, ExitStack() as stack:
        with tc.tile_pool(name="dram", bufs=1, space="DRAM") as dram:
            xt_d = reg(dram.tile([D, S], bf16, kind="ExternalInput", name="xt"), "xt")
            # fp8 DoubleRow weight blocks: [m-chunk, partition(k%128), kpair, j, out(128)]
            wq_d = reg(dram.tile([CH, P, KP, 2, P], fp8, kind="ExternalInput", name="wq"), "wq")
            wk_d = reg(dram.tile([CH, P, KP, 2, P], fp8, kind="ExternalInput", name="wk"), "wk")
            wo_d = reg(dram.tile([CH, P, KP, 2, P], fp8, kind="ExternalInput", name="wo"), "wo")
            wv_d = reg(dram.tile([CH, P, S], fp8, kind="ExternalInput", name="wv"), "wv")
            cw_d = reg(dram.tile([N_TAPS, CH, P, CH, P], bf16, kind="ExternalInput", name="cw"), "cw")
            uw_d = reg(dram.tile([8, CH, P, CH, P], bf16, kind="ExternalInput", name="uw"), "uw")
            bq_d = reg(dram.tile([P, CH], f32, kind="ExternalInput", name="bq"), "bq")
            bk_d = reg(dram.tile([P, CH], f32, kind="ExternalInput", name="bk"), "bk")
            bo_d = reg(dram.tile([P, CH], f32, kind="ExternalInput", name="bo"), "bo")
            bv_d = reg(dram.tile([1, S], f32, kind="ExternalInput", name="bv"), "bv")
            cb_d = reg(dram.tile([P, 3, CH], f32, kind="ExternalInput", name="cb"), "cb")
            yt_d = reg(dram.tile([D, S], bf16, kind="ExternalOutput", name="yt"), "yt")

            nc.gpsimd.load_library(library_config.attn)

            const = stack.enter_context(tc.tile_pool(name="const", bufs=1))
            ones32 = const.tile([P, 1], f32)
            nc.vector.memset(ones32[:], 1.0 / D)
            ones_b = const.tile([P, 1], bf16)
            nc.vector.tensor_copy(ones_b[:], ones32[:])
            bq_sb = const.tile([P, CH], f32)
            bk_sb = const.tile([P, CH], f32)
            bo_sb = const.tile([P, CH], f32)
            cb_sb = const.tile([P, 3, CH], f32)
            nc.sync.dma_start(bq_sb[:], bq_d[:])
            nc.sync.dma_start(bk_sb[:], bk_d[:])
            nc.sync.dma_start(bo_sb[:], bo_d[:])
            nc.sync.dma_start(cb_sb[:], cb_d[:])
            bv_row = const.tile([1, S], f32)
            nc.sync.dma_start(bv_row[:], bv_d[:])
            bv_b = const.tile([P, S], f32)
            nc.gpsimd.partition_broadcast(bv_b[:], bv_row[:])

            glob = stack.enter_context(tc.tile_pool(name="glob", bufs=1))
            x = glob.tile([P, CH, S], bf16, tag="x")          # residual x^T
            act8 = glob.tile([P, CH, S], fp8, tag="act8")     # LN1 output h (fp8)
            g = glob.tile([P, CH, S + 4], bf16, tag="g")      # LN2 output (padded)
            vw8 = glob.tile([P, CH, S], fp8, tag="vw8")       # V weights (resident)
            z = glob.tile([P, CH, 4, 512], bf16, tag="z")     # F(2,3) input transform
            for c in range(CH):
                nc.sync.dma_start(x[:, c, :], xt_d[c * P:(c + 1) * P, :])
            for c in range(CH):
                nc.sync.dma_start(vw8[:, c, :], wv_d[c])
            z16 = const.tile([P, 16], bf16)
            nc.vector.memset(z16[:], 0.0)
            zv = z16[:].rearrange("p (c e) -> p c e", e=2)
            nc.vector.tensor_copy(g[:, :, 0:2], zv)
            nc.vector.tensor_copy(g[:, :, S + 2:S + 4], zv)

            def emit_ln(dst, dst_off, sq_on_act=True):
                """LayerNorm over the feature (partition) axis of x.
                dst: act8 (fp8, attention input), g (bf16, conv input, +2 col
                offset), or None (final LN -> DMA f32 to yt_d)."""
                with tc.tile_pool(name="lnps", bufs=1, space="PSUM") as lnps, \
                     tc.tile_pool(name="lnsb", bufs=1) as lnsb:
                    mean_ps = lnps.tile([1, S], f32, tag="mean")
                    msq_ps = lnps.tile([1, S], f32, tag="msq")
                    for c in range(CH):
                        sq = lnsb.tile([P, S], bf16, tag="sq", bufs=2)
                        if sq_on_act:
                            nc.scalar.activation(sq[:], x[:, c, :], AF.Square)
                        else:
                            nc.vector.tensor_mul(sq[:], x[:, c, :], x[:, c, :])
                        for n in range(NH):
                            sl = slice(n * 512, (n + 1) * 512)
                            nc.tensor.matmul(mean_ps[:, sl], ones_b[:], x[:, c, sl],
                                             start=(c == 0), stop=(c == CH - 1))
                            nc.tensor.matmul(msq_ps[:, sl], ones_b[:], sq[:, sl],
                                             start=(c == 0), stop=(c == CH - 1))
                    lp = nc.allow_low_precision(reason="LN stats rows in bf16; 0.4% std err ok")
                    lp.__enter__()
                    mean_sb = lnsb.tile([1, S], bf16, tag="mrow")
                    nc.vector.tensor_copy(mean_sb[:], mean_ps[:])
                    m2 = lnsb.tile([1, S], bf16, tag="m2")
                    nc.vector.tensor_mul(m2[:], mean_sb[:], mean_sb[:])
                    var0 = lnsb.tile([1, S], bf16, tag="var0")
                    nc.vector.tensor_tensor(var0[:], msq_ps[:], m2[:], OP.subtract)
                    # unbiased std = sqrt(var0 * D/(D-1)); eps ~ 1e-6 is
                    # immaterial at our error scale.  Sqrt shares an ACT table
                    # set with Square/Identity/Relu (no table swap), and
                    # reciprocal runs on DVE.
                    stdr = lnsb.tile([1, S], bf16, tag="stdr")
                    nc.scalar.activation(stdr[:], var0[:], AF.Sqrt, scale=float(D / (D - 1.0)))
                    inv_rb = lnsb.tile([1, S], bf16, tag="invrb")
                    nc.vector.reciprocal(inv_rb[:], stdr[:])
                    minv_rb = lnsb.tile([1, S], bf16, tag="minvrb")
                    nc.vector.tensor_mul(minv_rb[:], mean_sb[:], inv_rb[:])
                    inv_b = lnsb.tile([P, S], bf16, tag="invb")
                    minv_b = lnsb.tile([P, S], bf16, tag="minvb")
                    nc.gpsimd.partition_broadcast(inv_b[:], inv_rb[:])
                    nc.gpsimd.partition_broadcast(minv_b[:], minv_rb[:])
                    lp.__exit__(None, None, None)
                    for c in range(CH):
                        t = lnsb.tile([P, S], bf16, tag="lnapp", bufs=2)
                        nc.vector.tensor_mul(t[:], x[:, c, :], inv_b[:])
                        if dst is not None:
                            nc.vector.tensor_tensor(dst[:, c, dst_off:dst_off + S],
                                                    t[:], minv_b[:], OP.subtract)
                        else:
                            o = lnsb.tile([P, S], bf16, tag="lnout", bufs=2)
                            nc.vector.tensor_tensor(o[:], t[:], minv_b[:], OP.subtract)
                            nc.sync.dma_start(yt_d[c * P:(c + 1) * P, :], o[:])

            for layer in range(NL):
                # ---------------- LN1 -> h (act8, fp8) ----------------
                emit_ln(act8, 0)

                with tc.tile_pool(name="attnbuf", bufs=1) as ab:
                    qt = ab.tile([P, CH, S], bf16, tag="qt")
                    kt = ab.tile([P, CH, S], bf16, tag="kt")
                    vt = ab.tile([P, CH, H * 65], bf16, tag="vt")
                    ot = ab.tile([P, CH, S], fp8, tag="ot")

                    # ---------------- Q,K projections (fp8 DoubleRow) --------
                    # Q/K interleaved per feature-chunk m so attention head
                    # pair 2m,2m+1 unblocks right after chunk m; Q drains on
                    # ACT, K on DVE (tensor_scalar) to split the drain load.
                    with tc.tile_pool(name="qkps", bufs=1, space="PSUM") as qkps, \
                         tc.tile_pool(name="qksb", bufs=1) as qksb:
                        for m in range(CH):
                            for (w_d, bsb, dst, on_act) in ((wq_d, bq_sb, qt, True),
                                                            (wk_d, bk_sb, kt, False)):
                                wblk = qksb.tile([P, KP, 2, P], fp8, tag="wproj", bufs=4)
                                nc.sync.dma_start(wblk[:], w_d[m])
                                for mb in range(2):
                                    msl = slice(mb * 64, (mb + 1) * 64)
                                    ps = qkps.tile([64, S], f32, tag=f"pps{mb}", bufs=1)
                                    for kp in range(KP):
                                        for n in range(NH):
                                            nc.tensor.matmul(
                                                ps[:, n * 512:(n + 1) * 512],
                                                wblk[:, kp, :, msl],
                                                act8[:, 2 * kp:2 * kp + 2, n * 512:(n + 1) * 512],
                                                start=(kp == 0), stop=(kp == KP - 1),
                                                perf_mode=PM.DoubleRow)
                                    if on_act:
                                        nc.scalar.activation(dst[msl, m, :], ps[:], AF.Identity,
                                                             bias=bsb[msl, m:m + 1], scale=1.0 / WSCALE)
                                    else:
                                        nc.vector.tensor_scalar(dst[msl, m, :], ps[:],
                                                                1.0 / WSCALE, bsb[msl, m:m + 1],
                                                                op0=OP.mult, op1=OP.add)

                        # ---------- V projection (fp8 DoubleRow, token-major)
                        nc.vector.memset(vt[:].rearrange("p c (h e) -> p c h e", e=65)[:, :, :, 64:65], 1.0)
                        for tb in range(2 * CH):
                            pv = qkps.tile([64, S], f32, tag="pv", bufs=2)
                            for kp in range(KP):
                                for n in range(NH):
                                    nc.tensor.matmul(
                                        pv[:, n * 512:(n + 1) * 512],
                                        act8[:, 2 * kp:2 * kp + 2, tb * 64:(tb + 1) * 64],
                                        vw8[:, 2 * kp:2 * kp + 2, n * 512:(n + 1) * 512],
                                        start=(kp == 0), stop=(kp == KP - 1),
                                        perf_mode=PM.DoubleRow)
                            p0 = (tb % 2) * 64
                            dstv = vt[p0:p0 + 64, tb // 2, :].rearrange("p (h e) -> p h e", e=65)[:, :, 0:64]
                            src = pv[:].rearrange("p (h e) -> p h e", e=64)
                            bvv = bv_b[0:64, :].rearrange("p (h e) -> p h e", e=64)
                            nc.vector.scalar_tensor_tensor(dstv, src, 1.0 / WSCALE, bvv,
                                                           op0=OP.mult, op1=OP.add)

                    # ---------------- attention per head ----------------
                    with tc.tile_pool(name="atps", bufs=1, space="PSUM") as atps, \
                         tc.tile_pool(name="atsb", bufs=1) as atsb:
                        for h in range(H):
                            hp, off = h // 2, 64 * (h % 2)
                            pexp = atsb.tile([P, CH, S], bf16, tag="pexp", bufs=2)
                            for mk in range(CH):
                                scps = atps.tile([P, S], f32, tag="scps", bufs=2)
                                for n in range(NH):
                                    nc.tensor.matmul(scps[:, n * 512:(n + 1) * 512],
                                                     kt[off:off + 64, hp, mk * P:(mk + 1) * P],
                                                     qt[off:off + 64, hp, n * 512:(n + 1) * 512],
                                                     start=True, stop=True)
                                nc.scalar.activation(pexp[:, mk, :], scps[:], AF.Exp)
                            ops = atps.tile([65, S], f32, tag="ops", bufs=2)
                            for c in range(CH):
                                for n in range(NH):
                                    sl = slice(n * 512, (n + 1) * 512)
                                    nc.tensor.matmul(ops[:, sl], vt[:, c, 65 * h:65 * h + 65],
                                                     pexp[:, c, sl],
                                                     start=(c == 0), stop=(c == CH - 1))
                            rrow = atsb.tile([1, S], f32, tag="rrow", bufs=2)
                            nc.vector.reciprocal(rrow[:], ops[64:65, :])
                            rb = atsb.tile([64, S], f32, tag="rb", bufs=2)
                            nc.gpsimd.partition_broadcast(rb[:], rrow[:])
                            nc.vector.tensor_tensor(ot[off:off + 64, hp, :], ops[0:64, :], rb[:], OP.mult)

                    # ---------------- output projection + residual ----------
                    with tc.tile_pool(name="wops", bufs=1, space="PSUM") as wops, \
                         tc.tile_pool(name="wosb", bufs=1) as wosb:
                        for m in range(CH):
                            woblk = wosb.tile([P, KP, 2, P], fp8, tag="wo", bufs=3)
                            nc.sync.dma_start(woblk[:], wo_d[m])
                            att = wosb.tile([P, S], bf16, tag="att", bufs=2)
                            for mb in range(2):
                                msl = slice(mb * 64, (mb + 1) * 64)
                                ps = wops.tile([64, S], f32, tag=f"ops{mb}", bufs=2)
                                for kp in range(KP):
                                    for n in range(NH):
                                        nc.tensor.matmul(
                                            ps[:, n * 512:(n + 1) * 512],
                                            woblk[:, kp, :, msl],
                                            ot[:, 2 * kp:2 * kp + 2, n * 512:(n + 1) * 512],
                                            start=(kp == 0), stop=(kp == KP - 1),
                                            perf_mode=PM.DoubleRow)
                                nc.scalar.activation(att[msl, :], ps[:], AF.Identity,
                                                     bias=bo_sb[msl, m:m + 1], scale=1.0 / WSCALE)
                            nc.vector.tensor_tensor(x[:, m, :], x[:, m, :], att[:], OP.add)

                # ---------------- LN2 -> g (bf16, padded) ----------------
                emit_ln(g, 2, sq_on_act=False)

                # F(2,3) input transform for the 3-tap branch: for output pair
                # (2u, 2u+1): d = g(2u-1 .. 2u+2); z0 = d0-d2, z1 = d1+d2,
                # z2 = d2-d1, z3 = d1-d3.  view(a) = g cols a+2t (t in 0..511).
                for c in range(CH):
                    view = lambda a: g[:, c, a:a + 1024].rearrange(
                        "p (t e) -> p e t", e=2)[:, 0, :]
                    nc.vector.tensor_tensor(z[:, c, 0, :], view(1), view(3), OP.subtract)
                    nc.vector.tensor_tensor(z[:, c, 1, :], view(2), view(3), OP.add)
                    nc.vector.tensor_tensor(z[:, c, 2, :], view(3), view(2), OP.subtract)
                    nc.vector.tensor_tensor(z[:, c, 3, :], view(2), view(4), OP.subtract)

                # ---------------- conv ensemble FFN (bf16) -------
                with tc.tile_pool(name="cvps", bufs=1, space="PSUM") as cvps, \
                     tc.tile_pool(name="cvsb", bufs=1) as cvsb:
                    def wino(wps, m, base, dst_even, dst_odd):
                        # phases m_i = U_i^T z_i; y_even = m0+m1+m2, y_odd = m1-m2-m3
                        for i in range(4):
                            uwblk = cvsb.tile([P, CH, P], bf16, tag="uw", bufs=6)
                            nc.sync.dma_start(uwblk[:], uw_d[base + i, m])
                            for c in range(CH):
                                nc.tensor.matmul(wps[:, i, :], uwblk[:, c, :],
                                                 z[:, c, i, :],
                                                 start=(c == 0), stop=(c == CH - 1))
                        # DVE may read only one PSUM operand per op: stage m1
                        m1 = cvsb.tile([P, 512], bf16, tag="m1s", bufs=2)
                        nc.vector.tensor_copy(m1[:], wps[:, 1, :])
                        te = cvsb.tile([P, 512], bf16, tag="te", bufs=2)
                        nc.vector.tensor_tensor(te[:], m1[:], wps[:, 0, :], OP.add)
                        to = cvsb.tile([P, 512], bf16, tag="to", bufs=2)
                        nc.vector.tensor_tensor(to[:], m1[:], wps[:, 2, :], OP.subtract)
                        nc.vector.tensor_tensor(dst_even, te[:], wps[:, 2, :], OP.add)
                        nc.vector.tensor_tensor(dst_odd, to[:], wps[:, 3, :], OP.subtract)

                    for m in range(CH):
                        # f5: outer taps (shifts +-2) direct into cps5
                        cps5 = cvps.tile([P, S], f32, tag="cps", bufs=2)
                        for ti, (tp, shift) in enumerate(((0, -2), (4, 2))):
                            cwblk = cvsb.tile([P, CH, P], bf16, tag="cw", bufs=4)
                            nc.sync.dma_start(cwblk[:], cw_d[tp, m])
                            for c in range(CH):
                                for n in range(NH):
                                    sl = slice(n * 512, (n + 1) * 512)
                                    u0 = 2 + shift + n * 512
                                    nc.tensor.matmul(cps5[:, sl], cwblk[:, c, :],
                                                     g[:, c, u0:u0 + 512],
                                                     start=(ti == 0 and c == 0),
                                                     stop=(ti == 1 and c == CH - 1))
                        # f5 inner 3 taps via Winograd -> w5 (interleaved)
                        w5 = cvsb.tile([P, S], bf16, tag="w5", bufs=2)
                        w5v = w5[:].rearrange("p (t e) -> p e t", e=2)
                        wps5 = cvps.tile([P, 4, 512], f32, tag="wps", bufs=1)
                        wino(wps5, m, 4, w5v[:, 0, :], w5v[:, 1, :])
                        # f1 direct (PE busy while f5 combine drains wps)
                        cps1 = cvps.tile([P, S], f32, tag="cps", bufs=2)
                        cwblk = cvsb.tile([P, CH, P], bf16, tag="cw", bufs=4)
                        nc.sync.dma_start(cwblk[:], cw_d[8, m])
                        for c in range(CH):
                            for n in range(NH):
                                sl = slice(n * 512, (n + 1) * 512)
                                nc.tensor.matmul(cps1[:, sl], cwblk[:, c, :],
                                                 g[:, c, 2 + n * 512:2 + n * 512 + 512],
                                                 start=(c == 0), stop=(c == CH - 1))
                        sc1 = cvsb.tile([P, S], bf16, tag="scr2", bufs=2)
                        nc.scalar.activation(sc1[:], cps1[:], AF.Relu, bias=cb_sb[:, 2, m:m + 1])
                        # f3 via Winograd -> sc3 (strided relu)
                        sc3 = cvsb.tile([P, S], bf16, tag="scr1", bufs=2)
                        ye = cvsb.tile([P, 512], bf16, tag="ye", bufs=2)
                        yo = cvsb.tile([P, 512], bf16, tag="yo", bufs=2)
                        wps3 = cvps.tile([P, 4, 512], f32, tag="wps", bufs=1)
                        wino(wps3, m, 0, ye[:], yo[:])
                        sc3v = sc3[:].rearrange("p (t e) -> p e t", e=2)
                        nc.scalar.activation(sc3v[:, 0, :], ye[:], AF.Relu,
                                             bias=cb_sb[:, 1, m:m + 1])
                        nc.scalar.activation(sc3v[:, 1, :], yo[:], AF.Relu,
                                             bias=cb_sb[:, 1, m:m + 1])
                        # f5 = relu(cps5 + w5 + bias)
                        y5 = cvsb.tile([P, S], bf16, tag="y5", bufs=2)
                        nc.vector.tensor_tensor(y5[:], cps5[:], w5[:], OP.add)
                        sc5 = cvsb.tile([P, S], bf16, tag="scr0", bufs=2)
                        nc.scalar.activation(sc5[:], y5[:], AF.Relu, bias=cb_sb[:, 0, m:m + 1])
                        t1 = cvsb.tile([P, S], bf16, tag="cmb", bufs=2)
                        nc.vector.tensor_tensor(t1[:], sc5[:], sc3[:], OP.add)
                        t2 = cvsb.tile([P, S], bf16, tag="cmb2", bufs=2)
                        nc.vector.tensor_tensor(t2[:], t1[:], sc1[:], OP.add)
                        nc.vector.tensor_tensor(x[:, m, :], x[:, m, :], t2[:], OP.add)

            # ---------------- final LN + writeback ----------------
            emit_ln(None, 0)

    nc.compile()
    return nc, names


_BUILT = None


def _get_built():
    global _BUILT
    if _BUILT is None:
        _BUILT = _build()
    return _BUILT


def _pack_fp8(w):
    # w: [D, D] contraction-major (w[k, o]) -> [CH_m, P, KP, 2, P] fp8 with
    # pk[m, p, kp, j, n] = w[(2*kp+j)*128 + p, 128*m + n] * WSCALE
    pk = (w * WSCALE).reshape(KP, 2, P, CH, P).transpose(3, 2, 0, 1, 4)
    return np.ascontiguousarray(pk).astype(ml_dtypes.float8_e4m3)


def _pack_lhsT(w):
    # w: [D, D] contraction-major -> [CH_m, P, CH_k, P] with pk[m,p,k,n] = w[128k+p, 128m+n]
    return np.ascontiguousarray(w.reshape(CH, P, CH, P).transpose(2, 1, 0, 3))


def _pack_bias(b):
    return np.ascontiguousarray(b.reshape(CH, P).T)


def _prep(inputs):
    f = lambda k: np.asarray(inputs[k], np.float32)
    a1, b1 = f('ln1_a'), f('ln1_b')
    a2, b2 = f('ln2_a'), f('ln2_b')
    wq, wk, wv, wo = f('wq'), f('wk'), f('wv'), f('wo')
    bq, bk, bv, bo = f('bq'), f('bk'), f('bv'), f('bo')

    d = {}
    d['wq'] = _pack_fp8((a1[:, None] * wq) / 8.0)
    d['bq'] = _pack_bias((bq + b1 @ wq) / 8.0)
    d['wk'] = _pack_fp8(a1[:, None] * wk)
    d['bk'] = _pack_bias(bk + b1 @ wk)
    d['wv'] = np.ascontiguousarray(
        (a1[:, None] * wv * WSCALE).reshape(CH, P, S)).astype(ml_dtypes.float8_e4m3)
    d['bv'] = (bv + b1 @ wv).reshape(1, S)
    d['wo'] = _pack_fp8(wo)
    d['bo'] = _pack_bias(bo)

    cw = np.empty((N_TAPS, CH, P, CH, P), ml_dtypes.bfloat16)
    uw = np.empty((8, CH, P, CH, P), ml_dtypes.bfloat16)
    cb = np.empty((P, 3, CH), np.float32)
    for bi, fs in enumerate((5, 3, 1)):
        i = 3 - bi   # conv_w1 is the 1-tap filter, conv_w3 the 5-tap one
        W = f(f'conv_w{i}')        # [oc, ic, f]
        b = f(f'conv_b{i}')
        g_, beta = f(f'bn_g{i}'), f(f'bn_b{i}')
        m, v = f(f'bn_m{i}'), f(f'bn_v{i}')
        s = g_ / np.sqrt(v + EPS_BN)
        Wf = W * s[:, None, None] * a2[None, :, None] / 3.0
        bias = ((b + W.sum(axis=2) @ b2 - m) * s + beta) / 3.0
        cb[:, bi, :] = _pack_bias(bias)
        if fs in (3, 5):
            j0 = 0 if fs == 3 else 1      # inner 3 taps (shifts -1,0,1)
            base = 0 if fs == 3 else 4
            W0, W1, W2 = Wf[:, :, j0], Wf[:, :, j0 + 1], Wf[:, :, j0 + 2]
            U = (W0, (W0 + W1 + W2) * 0.5, (W0 - W1 + W2) * 0.5, W2)
            for i in range(4):
                uw[base + i] = _pack_lhsT(np.ascontiguousarray(U[i].T)).astype(ml_dtypes.bfloat16)
        for j, (tp, _) in enumerate(BRANCH_TAPS[bi]):
            cw[tp] = _pack_lhsT(np.ascontiguousarray(Wf[:, :, j].T)).astype(ml_dtypes.bfloat16)
    d['cw'] = cw
    d['uw'] = uw
    d['cb'] = cb
    return d


def kernel(**inputs):
    nc, names = _get_built()
    shared = _prep(inputs)
    x = np.asarray(inputs['x'], np.float32)
    in_maps = []
    for b in range(N_CORES):
        m = {names[k]: v for k, v in shared.items()}
        m[names['xt']] = np.ascontiguousarray(x[b].T).astype(ml_dtypes.bfloat16)
        in_maps.append(m)
    res = run_bass_kernel_spmd(nc, in_maps, core_ids=list(range(N_CORES)))
    af = np.asarray(inputs['lnf_a'], np.float32)
    bf = np.asarray(inputs['lnf_b'], np.float32)
    out = np.empty((N_CORES, S, D), np.float32)
    for b in range(N_CORES):
        yt = np.asarray(res.results[b][names['yt']], np.float32)
        out[b] = yt.T * af[None, :] + bf[None, :]
    return out
